# revision 26
# baseline (speedup 1.0000x reference)
import sys

sys.path.insert(0, "/opt/trn_rl_repo")

import numpy as np

import concourse.bass as bass
import concourse.mybir as mybir
import concourse.tile as tile
from concourse import bacc
from concourse.bass_utils import run_bass_kernel_spmd
from concourse.masks import make_identity

# Problem dims (hardcoded per harness contract)
N, S, C = 4096, 1, 512
E, H, V = 64, 512, 256
T_STEPS = 32
M = 8            # cores
NL = N // M      # 512 rows per core
P = 128
KH = H // P      # 4 k-tiles over hidden dim
KV = V // P      # 2 k-tiles over vocab dim
NB = NL // P     # 4 batch tiles per core
HF = NL // 2     # 256-col half of the per-core batch

F32 = mybir.dt.float32
F32R = mybir.dt.float32r
BF16 = mybir.dt.bfloat16

_PROGRAM = None
LAST_RESULT = None


def _build_program():
    nc = bacc.Bacc("TRN2", target_bir_lowering=False, debug=False)

    whh_d = nc.dram_tensor("whh", [KH, P, 3 * H], F32R, kind="ExternalInput")
    wfold_d = nc.dram_tensor("wfold", [KV, P, 3 * H], F32R, kind="ExternalInput")
    gctxrz_d = nc.dram_tensor("gctxrz", [KH, P, 2, NL], F32, kind="ExternalInput")
    gctxn_d = nc.dram_tensor("gctxn", [KH, P, 2, NL], F32, kind="ExternalInput")
    fcwh_d = nc.dram_tensor("fcwh", [KH, P, V], F32R, kind="ExternalInput")
    fcwfold_d = nc.dram_tensor("fcwfold", [KV, P, V], F32R, kind="ExternalInput")
    lctx_d = nc.dram_tensor("lctx", [NB, P, V], F32, kind="ExternalInput")
    bhhn_d = nc.dram_tensor("bhhn", [P, KH], F32, kind="ExternalInput")
    oh0T_d = nc.dram_tensor("oh0T", [KV, P, NL], F32R, kind="ExternalInput")
    out_d = nc.dram_tensor("out", [NL, T_STEPS, V], F32, kind="ExternalOutput")

    Sig = mybir.ActivationFunctionType.Sigmoid
    Copy = mybir.ActivationFunctionType.Copy
    Tanh = mybir.ActivationFunctionType.Tanh
    ADD = mybir.AluOpType.add
    MULT = mybir.AluOpType.mult
    ISEQ = mybir.AluOpType.is_equal

    with tile.TileContext(nc) as tc:
        with tc.tile_pool(name="const", bufs=1) as const, \
             tc.tile_pool(name="state", bufs=2) as state, \
             tc.tile_pool(name="gate", bufs=2) as gate, \
             tc.tile_pool(name="work", bufs=4) as work, \
             tc.tile_pool(name="outp", bufs=2) as outp, \
             tc.tile_pool(name="rz", bufs=4, space="PSUM") as rzp, \
             tc.tile_pool(name="pl", bufs=1, space="PSUM") as plp, \
             tc.tile_pool(name="np", bufs=2, space="PSUM") as npp, \
             tc.tile_pool(name="pt", bufs=1, space="PSUM") as ptp:

            identb = const.tile([P, P], BF16)
            make_identity(nc, identb)

            # ---- constant loads (step-0 deps first) ----
            oh0T = const.tile([P, KV, NL], F32R)
            for k in range(KV):
                nc.sync.dma_start(out=oh0T[:, k, :], in_=oh0T_d[k])
            wfold = const.tile([P, KV, 3 * H], F32R)
            for k in range(KV):
                nc.sync.dma_start(out=wfold[:, k, :], in_=wfold_d[k])
            gctxrz = const.tile([P, KH, 2, NL], F32)
            for k in range(KH):
                nc.sync.dma_start(out=gctxrz[:, k, :, :], in_=gctxrz_d[k])
            gctxnz = const.tile([P, KH, 2, NL], F32)
            for k in range(KH):
                nc.sync.dma_start(out=gctxnz[:, k, :, :], in_=gctxn_d[k])
            bhhn = const.tile([P, KH], F32)
            nc.sync.dma_start(out=bhhn, in_=bhhn_d[:, :])
            lctx = const.tile([P, NB, V], F32)
            for k in range(NB):
                nc.sync.dma_start(out=lctx[:, k, :], in_=lctx_d[k])
            fcwh = const.tile([P, KH, V], F32R)
            for k in range(KH):
                nc.sync.dma_start(out=fcwh[:, k, :], in_=fcwh_d[k])
            fcwfold = const.tile([P, KV, V], F32R)
            for k in range(KV):
                nc.sync.dma_start(out=fcwfold[:, k, :], in_=fcwfold_d[k])
            whh = const.tile([P, KH, 3 * H], F32R)
            for k in range(KH):
                nc.sync.dma_start(out=whh[:, k, :], in_=whh_d[k])

            # cross-block tile references
            TL = {}

            def alloc_pl(t, q):
                # allocate + lctx-preload the logits psum for L(t, q)
                pl = plp.tile([P, 2, V], F32, tag="pl")
                TL[("pl", t, q)] = pl
                if t > 0:
                    nc.scalar.activation(pl[:, :, :],
                                         lctx[:, 2 * q:2 * q + 2, :],
                                         Copy, 0.0, 1.0)

            def G(t, q):
                qsl = slice(q * HF, (q + 1) * HF)
                if q == 0:
                    TL[("h", t)] = state.tile([P, KH, NL], F32R, tag="h", name="hT")
                    if t > 0:
                        TL[("oht", t)] = state.tile([P, KV, NL], F32R,
                                                    tag="oht", name="oht")
                    else:
                        TL[("oht", 0)] = oh0T
                    if t < T_STEPS - 1:
                        TL[("oh", t)] = gate.tile([P, NB, V], BF16, tag="oh", name="ohnv")
                        TL[("mx", t)] = work.tile([P, NB], F32, tag="mx", name="mx")
                hT_cur = TL[("h", t)]
                oht = TL[("oht", t)]
                rz_s = gate.tile([P, KH, 2, HF], F32, tag="rz")
                n_t = gate.tile([P, KH, HF], F32, tag="n")

                # Act: lctx preload for the L block that follows this G
                if q == 0:
                    if t > 0:
                        alloc_pl(t - 1, 1)
                else:
                    alloc_pl(t, 0)

                if t > 0:
                    hT_prev = TL[("h", t - 1)]
                    oh_prev = TL[("oh", t - 1)]

                    def trans():
                        pt = ptp.tile([P, NL], BF16, tag="pt")
                        for vb in range(KV):
                            for nb in range(2):
                                nc.tensor.transpose(
                                    pt[:, vb * HF + nb * P:
                                       vb * HF + (nb + 1) * P],
                                    oh_prev[:, nb + 2 * q,
                                            vb * P:(vb + 1) * P],
                                    identb)
                        for vb in range(KV):
                            nc.vector.tensor_copy(oht[:, vb, qsl],
                                                  pt[:, vb * HF:
                                                      (vb + 1) * HF])

                    # For q=0 the one-hot (iseq) is long ready; for q=1 it
                    # lands on DVE just before this block, so give it slack.
                    if q == 0:
                        trans()

                    # PE: r,z whh groups (psum preloaded with gctx by the
                    # previous block's Act stream)
                    RZ = TL[("RZ", t, q)]
                    for i in range(KH):
                        for j, m in ((0, i), (1, KH + i)):
                            msl = slice(m * P, (m + 1) * P)
                            for k in range(KH):
                                nc.tensor.matmul(RZ[i][:, j, :],
                                                 whh[:, k, msl],
                                                 hT_prev[:, k, qsl],
                                                 start=False, stop=False,
                                                 skip_group_check=True)
                        if q == 1 and i == 2:
                            trans()
                else:
                    RZ = []
                    for _ in range(KH):
                        rz0 = rzp.tile([P, 2, HF], F32, tag="rz", name="rz0")
                        nc.vector.memset(rz0[:, :, :], 0.0)
                        RZ.append(rz0)
                    TL[("RZ", 0, q)] = RZ

                # next G block in emission order (for preload targeting)
                nt, nq = (t, 1) if q == 0 else (t + 1, 0)
                do_next = 0 < nt < T_STEPS
                RZn = []

                # Per i: r,z wfold MMs, Sig, next-block gctx preload, then
                # the n-gate bank (px|pgh combined) and its DVE chain.  The
                # interleaving keeps every engine's FIFO free of waits on
                # later ops in the same queue.
                NPt = []

                def alloc_np(i):
                    # n-gate bank for i: col0 = px (gx+gctx), col1 = gh.
                    # Staggered so the DVE preload never queues behind the
                    # stt/nadd pair that frees its bank (bufs=2).
                    np_t = npp.tile([P, 2, HF], F32, tag="np", name="npt")
                    NPt.append(np_t)
                    if t > 0:
                        nc.vector.tensor_copy(np_t[:, :, :],
                                              gctxnz[:, i, :, qsl])
                    else:
                        nc.vector.memset(np_t[:, :, :], 0.0)

                alloc_np(0)
                alloc_np(1)
                for i in range(KH):
                    for j, m in ((0, i), (1, KH + i)):
                        msl = slice(m * P, (m + 1) * P)
                        nc.tensor.matmul(RZ[i][:, j, :],
                                         wfold[:, 0, msl], oht[:, 0, qsl],
                                         start=False, stop=False,
                                         skip_group_check=True)
                        nc.tensor.matmul(RZ[i][:, j, :],
                                         wfold[:, 1, msl], oht[:, 1, qsl],
                                         start=False, stop=True,
                                         skip_group_check=True)
                    if t > 0:
                        nc.scalar.activation(rz_s[:, i, :, :], RZ[i][:, :, :],
                                             Sig)
                    else:
                        nc.vector.tensor_add(rz_s[:, i, :, :], RZ[i][:, :, :],
                                             gctxrz[:, i, :, qsl])
                        nc.scalar.activation(rz_s[:, i, :, :],
                                             rz_s[:, i, :, :], Sig)
                    if do_next:
                        nqsl = slice(nq * HF, (nq + 1) * HF)
                        rzn = rzp.tile([P, 2, HF], F32, tag="rz")
                        RZn.append(rzn)
                        nc.scalar.activation(rzn[:, :, :],
                                             gctxrz[:, i, :, nqsl],
                                             Copy, 0.0, 1.0)

                for i in range(KH):
                    np_t = NPt[i]
                    pxv = np_t[:, 0, :]
                    msl = slice((2 * KH + i) * P, (2 * KH + i + 1) * P)
                    if t > 0:
                        pghv = np_t[:, 1, :]
                        for k in range(KH):
                            nc.tensor.matmul(pghv, whh[:, k, msl],
                                             hT_prev[:, k, qsl],
                                             start=False,
                                             stop=(k == KH - 1),
                                             skip_group_check=True)
                    nc.tensor.matmul(pxv, wfold[:, 0, msl], oht[:, 0, qsl],
                                     start=False, stop=False,
                                     skip_group_check=True)
                    nc.tensor.matmul(pxv, wfold[:, 1, msl], oht[:, 1, qsl],
                                     start=False, stop=True,
                                     skip_group_check=True)

                    u = work.tile([P, HF], F32, tag="u")
                    if t > 0:
                        nc.vector.scalar_tensor_tensor(
                            u, np_t[:, 1, :], bhhn[:, i:i + 1],
                            rz_s[:, i, 0, :], ADD, MULT)
                    else:
                        nc.vector.tensor_scalar(u, rz_s[:, i, 0, :],
                                                bhhn[:, i:i + 1],
                                                None, MULT)
                    nc.vector.tensor_add(n_t[:, i, :], pxv, u)
                    if t == 0:
                        nc.vector.tensor_add(n_t[:, i, :], n_t[:, i, :],
                                             gctxnz[:, i, 0, qsl])
                    if i + 2 < KH:
                        alloc_np(i + 2)
                if do_next:
                    TL[("RZ", nt, nq)] = RZn

                # trailing chain: tanh + h update.  The last i-tile's
                # update runs on DVE (lower per-op latency) — its h gates
                # the next step's whh burst.
                for i in range(KH):
                    nc.scalar.activation(n_t[:, i, :], n_t[:, i, :], Tanh)
                    v = work.tile([P, HF], F32, tag="v")
                    if t > 0 and i == KH - 1:
                        nc.vector.tensor_sub(v, hT_prev[:, i, qsl],
                                             n_t[:, i, :])
                        nc.vector.tensor_mul(v, v, rz_s[:, i, 1, :])
                        nc.vector.tensor_add(hT_cur[:, i, qsl], v,
                                             n_t[:, i, :])
                    elif t > 0:
                        nc.gpsimd.tensor_sub(v, hT_prev[:, i, qsl],
                                             n_t[:, i, :])
                        nc.gpsimd.tensor_mul(v, v, rz_s[:, i, 1, :])
                        nc.gpsimd.tensor_add(hT_cur[:, i, qsl], v,
                                             n_t[:, i, :])
                    else:
                        nc.vector.tensor_scalar(v, rz_s[:, i, 1, :],
                                                -1.0, 1.0, MULT, ADD)
                        nc.vector.tensor_mul(hT_cur[:, i, qsl], v,
                                             n_t[:, i, :])

            def L(t, q):
                oht = TL[("oht", t)]
                hT_cur = TL[("h", t)]
                if t == T_STEPS - 1 and q == 1:
                    alloc_pl(T_STEPS - 1, 1)
                pl = TL[("pl", t, q)]
                if t == 0:
                    nc.vector.memset(pl[:, :, :], 0.0)
                lg = outp.tile([P, 2, V], F32, tag="lg")
                for j, nb in enumerate((2 * q, 2 * q + 1)):
                    nsl = slice(nb * P, (nb + 1) * P)
                    plv = pl[:, j, :]
                    nc.tensor.matmul(plv, oht[:, 0, nsl], fcwfold[:, 0, :],
                                     start=False, stop=False,
                                     skip_group_check=True)
                    nc.tensor.matmul(plv, oht[:, 1, nsl], fcwfold[:, 1, :],
                                     start=False, stop=False,
                                     skip_group_check=True)
                    for k in range(KH):
                        nc.tensor.matmul(plv, hT_cur[:, k, nsl],
                                         fcwh[:, k, :],
                                         start=False, stop=(k == KH - 1),
                                         skip_group_check=True)
                if t > 0:
                    nc.scalar.activation(lg[:, :, :], pl[:, :, :],
                                         Copy, 0.0, 1.0)
                else:
                    nc.vector.tensor_add(lg[:, :, :], pl[:, :, :],
                                         lctx[:, 2 * q:2 * q + 2, :])
                for j, nb in enumerate((2 * q, 2 * q + 1)):
                    nsl = slice(nb * P, (nb + 1) * P)
                    nc.sync.dma_start(out=out_d[nsl, t, :], in_=lg[:, j, :])
                if t < T_STEPS - 1:
                    mx = TL[("mx", t)]
                    oh_nv = TL[("oh", t)]
                    for j, nb in enumerate((2 * q, 2 * q + 1)):
                        amsrc = pl[:, j, :] if t > 0 else lg[:, j, :]
                        nc.vector.tensor_reduce(out=mx[:, nb:nb + 1],
                                                in_=amsrc,
                                                axis=mybir.AxisListType.X,
                                                op=mybir.AluOpType.max)
                        nc.vector.tensor_scalar(oh_nv[:, nb, :], amsrc,
                                                mx[:, nb:nb + 1], None,
                                                ISEQ)

            for t in range(T_STEPS):
                G(t, 0)
                if t > 0:
                    L(t - 1, 1)
                G(t, 1)
                L(t, 0)
            L(T_STEPS - 1, 1)

    nc.compile()
    return nc


def _get_program():
    global _PROGRAM
    if _PROGRAM is None:
        _PROGRAM = _build_program()
    return _PROGRAM


def kernel(encoded, init_token, emb_W, W_ih, W_hh, b_ih, b_hh, fc_W, fc_b, T):
    global LAST_RESULT
    assert int(T) == T_STEPS
    encoded = np.asarray(encoded, np.float64)
    init_token = np.asarray(init_token).astype(np.int64)
    emb_W = np.asarray(emb_W, np.float64)
    W_ih = np.asarray(W_ih, np.float64)
    W_hh = np.asarray(W_hh, np.float64)
    b_ih = np.asarray(b_ih, np.float64)
    b_hh = np.asarray(b_hh, np.float64)
    fc_W = np.asarray(fc_W, np.float64)
    fc_b = np.asarray(fc_b, np.float64)

    cx = np.ascontiguousarray

    # shared weights
    whh = cx(W_hh.T.reshape(KH, P, 3 * H).astype(np.float32))
    wfold = cx((W_ih[:, :E] @ emb_W.T).T.reshape(KV, P, 3 * H).astype(np.float32))
    fcwh = cx(fc_W[:, E + C:].T.reshape(KH, P, V).astype(np.float32))
    fcwfold = cx((fc_W[:, :E] @ emb_W.T).T.reshape(KV, P, V).astype(np.float32))
    bhhn = cx(b_hh[2 * H:].reshape(KH, P).T.astype(np.float32))

    # context GEMMs precomputed exactly on host (fp64)
    ctx_all = encoded.reshape(N, C)
    bias_g = b_ih.copy()
    bias_g[:2 * H] += b_hh[:2 * H]
    gctx_all = ctx_all @ W_ih[:, E:].T + bias_g          # [N, 3H]
    lctx_all = ctx_all @ fc_W[:, E:E + C].T + fc_b       # [N, V]

    in_maps = []
    for c in range(M):
        sl = slice(c * NL, (c + 1) * NL)
        gctx = gctx_all[sl].T.reshape(3 * H // P, P, NL).astype(np.float32)
        # r|z interleaved: [i, p, 0, n] = r-row i*128+p, [i, p, 1, n] = z-row
        gctxrz = np.empty((KH, P, 2, NL), np.float32)
        for i in range(KH):
            gctxrz[i, :, 0, :] = gctx[i]
            gctxrz[i, :, 1, :] = gctx[KH + i]
        gctxrz = cx(gctxrz)
        gctxnz = np.zeros((KH, P, 2, NL), np.float32)
        gctxnz[:, :, 0, :] = gctx[2 * KH:]
        gctxnz = cx(gctxnz)
        lctxc = cx(lctx_all[sl].reshape(NB, P, V).astype(np.float32))
        oh = np.zeros((V, NL), np.float32)
        oh[init_token[sl], np.arange(NL)] = 1.0
        oh0T = cx(oh.reshape(KV, P, NL))
        in_maps.append({
            "whh": whh, "wfold": wfold, "gctxrz": gctxrz, "gctxn": gctxnz,
            "fcwh": fcwh, "fcwfold": fcwfold, "lctx": lctxc, "bhhn": bhhn,
            "oh0T": oh0T,
        })

    nc = _get_program()
    res = run_bass_kernel_spmd(nc, in_maps, core_ids=list(range(M)))
    LAST_RESULT = res
    out = np.empty((N, T_STEPS, V), np.float32)
    for c in range(M):
        out[c * NL:(c + 1) * NL] = res.results[c]["out"]
    return out


# revision 27
# speedup vs baseline: 1.3910x; 1.3910x over previous
import sys

sys.path.insert(0, "/opt/trn_rl_repo")

import numpy as np

import concourse.bass as bass
import concourse.mybir as mybir
import concourse.tile as tile
from concourse import bacc
from concourse.bass_utils import run_bass_kernel_spmd
from concourse.masks import make_identity

# Problem dims (hardcoded per harness contract)
N, S, C = 4096, 1, 512
E, H, V = 64, 512, 256
T_STEPS = 32
M = 8            # cores
NL = N // M      # 512 rows per core
P = 128
KH = H // P      # 4 k-tiles over hidden dim
KV = V // P      # 2 k-tiles over vocab dim
NB = NL // P     # 4 batch tiles per core
HF = NL // 2     # 256-col half of the per-core batch

F32 = mybir.dt.float32
F32R = mybir.dt.float32r
BF16 = mybir.dt.bfloat16

_PROGRAM = None
LAST_RESULT = None


def _build_program():
    nc = bacc.Bacc("TRN2", target_bir_lowering=False, debug=False)

    whh_d = nc.dram_tensor("whh", [KH, P, 3 * H], F32R, kind="ExternalInput")
    wfold_d = nc.dram_tensor("wfold", [KV, P, 3 * H], F32R, kind="ExternalInput")
    gctxrz_d = nc.dram_tensor("gctxrz", [KH, P, 2, NL], F32, kind="ExternalInput")
    gctxn_d = nc.dram_tensor("gctxn", [KH, P, 2, NL], F32, kind="ExternalInput")
    fcwh_d = nc.dram_tensor("fcwh", [KH, P, V], F32R, kind="ExternalInput")
    fcwfold_d = nc.dram_tensor("fcwfold", [KV, P, V], F32R, kind="ExternalInput")
    lctx_d = nc.dram_tensor("lctx", [NB, P, V], F32, kind="ExternalInput")
    bhhn_d = nc.dram_tensor("bhhn", [P, KH], F32, kind="ExternalInput")
    oh0T_d = nc.dram_tensor("oh0T", [KV, P, NL], F32R, kind="ExternalInput")
    out_d = nc.dram_tensor("out", [NL, T_STEPS, V], F32, kind="ExternalOutput")

    Sig = mybir.ActivationFunctionType.Sigmoid
    Copy = mybir.ActivationFunctionType.Copy
    Tanh = mybir.ActivationFunctionType.Tanh
    ADD = mybir.AluOpType.add
    MULT = mybir.AluOpType.mult
    ISEQ = mybir.AluOpType.is_equal

    with tile.TileContext(nc) as tc:
        with tc.tile_pool(name="const", bufs=1) as const, \
             tc.tile_pool(name="state", bufs=2) as state, \
             tc.tile_pool(name="gate", bufs=2) as gate, \
             tc.tile_pool(name="work", bufs=4) as work, \
             tc.tile_pool(name="outp", bufs=2) as outp, \
             tc.tile_pool(name="rz", bufs=4, space="PSUM") as rzp, \
             tc.tile_pool(name="pl", bufs=1, space="PSUM") as plp, \
             tc.tile_pool(name="np", bufs=2, space="PSUM") as npp, \
             tc.tile_pool(name="pt", bufs=1, space="PSUM") as ptp:

            identb = const.tile([P, P], BF16)
            make_identity(nc, identb)

            # ---- constant loads (step-0 deps first) ----
            oh0T = const.tile([P, KV, NL], F32R)
            for k in range(KV):
                nc.sync.dma_start(out=oh0T[:, k, :], in_=oh0T_d[k])
            wfold = const.tile([P, KV, 3 * H], F32R)
            for k in range(KV):
                nc.sync.dma_start(out=wfold[:, k, :], in_=wfold_d[k])
            gctxrz = const.tile([P, KH, 2, NL], F32)
            for k in range(KH):
                nc.sync.dma_start(out=gctxrz[:, k, :, :], in_=gctxrz_d[k])
            gctxnz = const.tile([P, KH, 2, NL], F32)
            for k in range(KH):
                nc.sync.dma_start(out=gctxnz[:, k, :, :], in_=gctxn_d[k])
            bhhn = const.tile([P, KH], F32)
            nc.sync.dma_start(out=bhhn, in_=bhhn_d[:, :])
            lctx = const.tile([P, NB, V], F32)
            for k in range(NB):
                nc.sync.dma_start(out=lctx[:, k, :], in_=lctx_d[k])
            fcwh = const.tile([P, KH, V], F32R)
            for k in range(KH):
                nc.sync.dma_start(out=fcwh[:, k, :], in_=fcwh_d[k])
            fcwfold = const.tile([P, KV, V], F32R)
            for k in range(KV):
                nc.sync.dma_start(out=fcwfold[:, k, :], in_=fcwfold_d[k])
            whh = const.tile([P, KH, 3 * H], F32R)
            for k in range(KH):
                nc.sync.dma_start(out=whh[:, k, :], in_=whh_d[k])

            # cross-block tile references
            TL = {}

            def alloc_pl(t, q):
                # allocate + lctx-preload the logits psum for L(t, q)
                pl = plp.tile([P, 2, V], F32, tag="pl")
                TL[("pl", t, q)] = pl
                if t > 0:
                    nc.scalar.activation(pl[:, :, :],
                                         lctx[:, 2 * q:2 * q + 2, :],
                                         Copy, 0.0, 1.0)

            def G(t, q):
                qsl = slice(q * HF, (q + 1) * HF)
                if q == 0:
                    TL[("h", t)] = state.tile([P, KH, NL], F32R, tag="h", name="hT")
                    if t > 0:
                        TL[("oht", t)] = state.tile([P, KV, NL], F32R,
                                                    tag="oht", name="oht")
                    else:
                        TL[("oht", 0)] = oh0T
                    if t < T_STEPS - 1:
                        TL[("oh", t)] = gate.tile([P, NB, V], BF16, tag="oh", name="ohnv")
                        TL[("mx", t)] = work.tile([P, NB], F32, tag="mx", name="mx")
                hT_cur = TL[("h", t)]
                oht = TL[("oht", t)]
                rz_s = gate.tile([P, KH, 2, HF], F32, tag="rz")
                n_t = gate.tile([P, KH, HF], F32, tag="n")

                # Act: lctx preload for the L block that follows this G
                if q == 0:
                    if t > 0:
                        alloc_pl(t - 1, 1)
                else:
                    alloc_pl(t, 0)

                if t > 0:
                    hT_prev = TL[("h", t - 1)]
                    oh_prev = TL[("oh", t - 1)]

                    def trans():
                        pt = ptp.tile([P, NL], BF16, tag="pt")
                        for vb in range(KV):
                            for nb in range(2):
                                nc.tensor.transpose(
                                    pt[:, vb * HF + nb * P:
                                       vb * HF + (nb + 1) * P],
                                    oh_prev[:, nb + 2 * q,
                                            vb * P:(vb + 1) * P],
                                    identb)
                        for vb in range(KV):
                            nc.vector.tensor_copy(oht[:, vb, qsl],
                                                  pt[:, vb * HF:
                                                      (vb + 1) * HF])

                    # For q=0 the one-hot (iseq) is long ready; for q=1 it
                    # lands on DVE just before this block, so give it slack.
                    if q == 0:
                        trans()

                    # PE: r,z whh groups (psum preloaded with gctx by the
                    # previous block's Act stream)
                    RZ = TL[("RZ", t, q)]
                    for i in range(KH):
                        for j, m in ((0, i), (1, KH + i)):
                            msl = slice(m * P, (m + 1) * P)
                            for k in range(KH):
                                nc.tensor.matmul(RZ[i][:, j, :],
                                                 whh[:, k, msl],
                                                 hT_prev[:, k, qsl],
                                                 start=False, stop=False,
                                                 skip_group_check=True)
                        if q == 1 and i == 1:
                            trans()
                else:
                    RZ = []
                    for _ in range(KH):
                        rz0 = rzp.tile([P, 2, HF], F32, tag="rz", name="rz0")
                        nc.vector.memset(rz0[:, :, :], 0.0)
                        RZ.append(rz0)
                    TL[("RZ", 0, q)] = RZ

                # next G block in emission order (for preload targeting)
                nt, nq = (t, 1) if q == 0 else (t + 1, 0)
                do_next = 0 < nt < T_STEPS
                RZn = []

                # Per i: r,z wfold MMs, Sig, next-block gctx preload, then
                # the n-gate bank (px|pgh combined) and its DVE chain.  The
                # interleaving keeps every engine's FIFO free of waits on
                # later ops in the same queue.
                NPt = []

                def alloc_np(i):
                    # n-gate bank for i: col0 = px (gx+gctx), col1 = gh.
                    # Staggered so the DVE preload never queues behind the
                    # stt/nadd pair that frees its bank (bufs=2).
                    np_t = npp.tile([P, 2, HF], F32, tag="np", name="npt")
                    NPt.append(np_t)
                    if t > 0:
                        nc.vector.tensor_copy(np_t[:, :, :],
                                              gctxnz[:, i, :, qsl])
                    else:
                        nc.vector.memset(np_t[:, :, :], 0.0)

                alloc_np(0)
                alloc_np(1)
                for i in range(KH):
                    for j, m in ((0, i), (1, KH + i)):
                        msl = slice(m * P, (m + 1) * P)
                        if t > 0:
                            nc.tensor.matmul(RZ[i][:, j, :],
                                             wfold[:, 0, msl], oht[:, 0, qsl],
                                             start=False, stop=False,
                                             skip_group_check=True)
                            nc.tensor.matmul(RZ[i][:, j, :],
                                             wfold[:, 1, msl], oht[:, 1, qsl],
                                             start=False, stop=True,
                                             skip_group_check=True)
                        else:
                            nc.tensor.matmul(RZ[i][:, j, :],
                                             wfold[:, 0, msl], oht[:, 0, qsl],
                                             start=False, stop=False,
                                             skip_group_check=True)
                            nc.tensor.matmul(RZ[i][:, j, :],
                                             wfold[:, 1, msl], oht[:, 1, qsl],
                                             start=False, stop=True,
                                             skip_group_check=True)
                    if t > 0:
                        nc.scalar.activation(rz_s[:, i, :, :], RZ[i][:, :, :],
                                             Sig)
                    else:
                        nc.vector.tensor_add(rz_s[:, i, :, :], RZ[i][:, :, :],
                                             gctxrz[:, i, :, qsl])
                        nc.scalar.activation(rz_s[:, i, :, :],
                                             rz_s[:, i, :, :], Sig)
                    if do_next:
                        nqsl = slice(nq * HF, (nq + 1) * HF)
                        rzn = rzp.tile([P, 2, HF], F32, tag="rz")
                        RZn.append(rzn)
                        nc.scalar.activation(rzn[:, :, :],
                                             gctxrz[:, i, :, nqsl],
                                             Copy, 0.0, 1.0)

                    np_t = NPt[i]
                    pxv = np_t[:, 0, :]
                    msl = slice((2 * KH + i) * P, (2 * KH + i + 1) * P)
                    if t > 0:
                        pghv = np_t[:, 1, :]
                        for k in range(KH):
                            nc.tensor.matmul(pghv, whh[:, k, msl],
                                             hT_prev[:, k, qsl],
                                             start=False,
                                             stop=(k == KH - 1),
                                             skip_group_check=True)
                    nc.tensor.matmul(pxv, wfold[:, 0, msl], oht[:, 0, qsl],
                                     start=False, stop=False,
                                     skip_group_check=True)
                    nc.tensor.matmul(pxv, wfold[:, 1, msl], oht[:, 1, qsl],
                                     start=False, stop=True,
                                     skip_group_check=True)

                    u = work.tile([P, HF], F32, tag="u")
                    if t > 0:
                        nc.vector.scalar_tensor_tensor(
                            u, np_t[:, 1, :], bhhn[:, i:i + 1],
                            rz_s[:, i, 0, :], ADD, MULT)
                    else:
                        nc.vector.tensor_scalar(u, rz_s[:, i, 0, :],
                                                bhhn[:, i:i + 1],
                                                None, MULT)
                    nc.vector.tensor_add(n_t[:, i, :], pxv, u)
                    if t == 0:
                        nc.vector.tensor_add(n_t[:, i, :], n_t[:, i, :],
                                             gctxnz[:, i, 0, qsl])
                    if i + 2 < KH:
                        alloc_np(i + 2)
                if do_next:
                    TL[("RZ", nt, nq)] = RZn

                # trailing chain: tanh + h update
                for i in range(KH):
                    nc.scalar.activation(n_t[:, i, :], n_t[:, i, :], Tanh)
                    v = work.tile([P, HF], F32, tag="v")
                    if t > 0:
                        nc.gpsimd.tensor_sub(v, hT_prev[:, i, qsl],
                                             n_t[:, i, :])
                        nc.gpsimd.tensor_mul(v, v, rz_s[:, i, 1, :])
                        nc.gpsimd.tensor_add(hT_cur[:, i, qsl], v,
                                             n_t[:, i, :])
                    else:
                        nc.vector.tensor_scalar(v, rz_s[:, i, 1, :],
                                                -1.0, 1.0, MULT, ADD)
                        nc.vector.tensor_mul(hT_cur[:, i, qsl], v,
                                             n_t[:, i, :])

            def L(t, q):
                oht = TL[("oht", t)]
                hT_cur = TL[("h", t)]
                if t == T_STEPS - 1 and q == 1:
                    alloc_pl(T_STEPS - 1, 1)
                pl = TL[("pl", t, q)]
                if t == 0:
                    nc.vector.memset(pl[:, :, :], 0.0)
                lg = outp.tile([P, 2, V], F32, tag="lg")
                for j, nb in enumerate((2 * q, 2 * q + 1)):
                    nsl = slice(nb * P, (nb + 1) * P)
                    plv = pl[:, j, :]
                    nc.tensor.matmul(plv, oht[:, 0, nsl], fcwfold[:, 0, :],
                                     start=False, stop=False,
                                     skip_group_check=True)
                    nc.tensor.matmul(plv, oht[:, 1, nsl], fcwfold[:, 1, :],
                                     start=False, stop=False,
                                     skip_group_check=True)
                    for k in range(KH):
                        nc.tensor.matmul(plv, hT_cur[:, k, nsl],
                                         fcwh[:, k, :],
                                         start=False, stop=(k == KH - 1),
                                         skip_group_check=True)
                if t > 0:
                    nc.scalar.activation(lg[:, :, :], pl[:, :, :],
                                         Copy, 0.0, 1.0)
                else:
                    nc.vector.tensor_add(lg[:, :, :], pl[:, :, :],
                                         lctx[:, 2 * q:2 * q + 2, :])
                for j, nb in enumerate((2 * q, 2 * q + 1)):
                    nsl = slice(nb * P, (nb + 1) * P)
                    nc.sync.dma_start(out=out_d[nsl, t, :], in_=lg[:, j, :])
                if t < T_STEPS - 1:
                    mx = TL[("mx", t)]
                    oh_nv = TL[("oh", t)]
                    for j, nb in enumerate((2 * q, 2 * q + 1)):
                        amsrc = pl[:, j, :] if t > 0 else lg[:, j, :]
                        nc.vector.tensor_reduce(out=mx[:, nb:nb + 1],
                                                in_=amsrc,
                                                axis=mybir.AxisListType.X,
                                                op=mybir.AluOpType.max)
                        nc.vector.tensor_scalar(oh_nv[:, nb, :], amsrc,
                                                mx[:, nb:nb + 1], None,
                                                ISEQ)

            for t in range(T_STEPS):
                G(t, 0)
                if t > 0:
                    L(t - 1, 1)
                G(t, 1)
                L(t, 0)
            L(T_STEPS - 1, 1)

    nc.compile()
    return nc


def _get_program():
    global _PROGRAM
    if _PROGRAM is None:
        _PROGRAM = _build_program()
    return _PROGRAM


def kernel(encoded, init_token, emb_W, W_ih, W_hh, b_ih, b_hh, fc_W, fc_b, T):
    global LAST_RESULT
    assert int(T) == T_STEPS
    encoded = np.asarray(encoded, np.float64)
    init_token = np.asarray(init_token).astype(np.int64)
    emb_W = np.asarray(emb_W, np.float64)
    W_ih = np.asarray(W_ih, np.float64)
    W_hh = np.asarray(W_hh, np.float64)
    b_ih = np.asarray(b_ih, np.float64)
    b_hh = np.asarray(b_hh, np.float64)
    fc_W = np.asarray(fc_W, np.float64)
    fc_b = np.asarray(fc_b, np.float64)

    cx = np.ascontiguousarray

    # shared weights
    whh = cx(W_hh.T.reshape(KH, P, 3 * H).astype(np.float32))
    wfold = cx((W_ih[:, :E] @ emb_W.T).T.reshape(KV, P, 3 * H).astype(np.float32))
    fcwh = cx(fc_W[:, E + C:].T.reshape(KH, P, V).astype(np.float32))
    fcwfold = cx((fc_W[:, :E] @ emb_W.T).T.reshape(KV, P, V).astype(np.float32))
    bhhn = cx(b_hh[2 * H:].reshape(KH, P).T.astype(np.float32))

    # context GEMMs precomputed exactly on host (fp64)
    ctx_all = encoded.reshape(N, C)
    bias_g = b_ih.copy()
    bias_g[:2 * H] += b_hh[:2 * H]
    gctx_all = ctx_all @ W_ih[:, E:].T + bias_g          # [N, 3H]
    lctx_all = ctx_all @ fc_W[:, E:E + C].T + fc_b       # [N, V]

    in_maps = []
    for c in range(M):
        sl = slice(c * NL, (c + 1) * NL)
        gctx = gctx_all[sl].T.reshape(3 * H // P, P, NL).astype(np.float32)
        # r|z interleaved: [i, p, 0, n] = r-row i*128+p, [i, p, 1, n] = z-row
        gctxrz = np.empty((KH, P, 2, NL), np.float32)
        for i in range(KH):
            gctxrz[i, :, 0, :] = gctx[i]
            gctxrz[i, :, 1, :] = gctx[KH + i]
        gctxrz = cx(gctxrz)
        gctxnz = np.zeros((KH, P, 2, NL), np.float32)
        gctxnz[:, :, 0, :] = gctx[2 * KH:]
        gctxnz = cx(gctxnz)
        lctxc = cx(lctx_all[sl].reshape(NB, P, V).astype(np.float32))
        oh = np.zeros((V, NL), np.float32)
        oh[init_token[sl], np.arange(NL)] = 1.0
        oh0T = cx(oh.reshape(KV, P, NL))
        in_maps.append({
            "whh": whh, "wfold": wfold, "gctxrz": gctxrz, "gctxn": gctxnz,
            "fcwh": fcwh, "fcwfold": fcwfold, "lctx": lctxc, "bhhn": bhhn,
            "oh0T": oh0T,
        })

    nc = _get_program()
    res = run_bass_kernel_spmd(nc, in_maps, core_ids=list(range(M)))
    LAST_RESULT = res
    out = np.empty((N, T_STEPS, V), np.float32)
    for c in range(M):
        out[c * NL:(c + 1) * NL] = res.results[c]["out"]
    return out
